# revision 19
# baseline (speedup 1.0000x reference)
"""2-layer GCN (PyG GCNConv x2) on 8 TRN2 NeuronCores via Bass/Tile.

Sharding: nodes (rows of x and of the segment-sum output) are sharded across
the 8 cores; the small weight matrices are replicated. Edge messages are
routed to the core owning their dst node (host-side index prep). Each layer:
  y = dinv * (x @ W)            (own shard, PE matmuls, bf16)
  AllGather y -> y_full         (chunked collective, Shared-HBM output)
  z[d] = sum_{(s,d) in E} y[s]  (dma_gather rows + one-hot matmul segment-sum)
  out[d] = dinv[d]*z[d] + b     (+ ReLU for layer 1)

Self-loops are folded into the edge list host-side (so no separate identity
matmul). The gathered table is bf16 with 128-wide rows (256 B). Gather
indexing: nodes are split into 4 classes by table QUARTER (contiguous 25600
phys-row ranges, int16-addressable), so each gather call reads one quarter
and only depends on the 2 AllGather chunks covering it. Gather calls are
merged across groups of G=4 dst blocks (one call per (group, class), 4 SWDGE
queues) to amortize the ~1-2.4us per-call Pool-engine descriptor-gen cost.
Each (block, class) run inside a call is 128-row aligned (pad indices point
at row 0 of the quarter; their seg ids are PAD so the one-hot excludes them).
The table uses a chunk-major physical layout (8 chunks of 8-core row ranges,
quarter boundaries at even chunks) so each chunked AllGather writes one
contiguous range. The per-block one-hot matrices for the segment-sum matmuls
are built in ONE fused DVE is_equal per block via 3-level broadcast APs.
"""
import math
import numpy as np
import ml_dtypes

import concourse.bass as bass
import concourse.tile as tile
from concourse import bacc, mybir
from concourse.bass_utils import run_bass_kernel_spmd

P = 128
NCORES = 8
N_NODES = 100000
NPAD = 102400            # 8 * 12800
SH = NPAD // NCORES      # 12800 rows per core
NB = SH // P             # 100 blocks of 128 dst rows
NCLS = 4                 # table-quarter classes (int16 index range)
QB = NPAD // NCLS        # 25600 phys rows per quarter
F1, F2, F3 = 300, 128, 64
TW = 128                 # gathered-table row width (bf16 -> 256B rows)
PAD_SEG = 255.0
BF16 = mybir.dt.bfloat16
AGB = [0, 25, 50, 75, 100]    # AllGather chunk bounds == quarter bounds, so
CUMB = [b * P for b in AGB]   # the collective's core-major concat matches phys
G = 4                    # dst blocks per merged gather call
NG = NB // G             # 25 groups
GA = 4                   # stage-A blocks sharing one x-chunk DMA
SHARED = True            # quarter tables in Shared HBM (one writer each)


def _build_program(cnk, moff, chg, ioff, soff, ni16_cols, chtot_seg):
    """Build the shared 8-core Bass program.

    cnk[b][c]   = 128-row chunks for (dst block b, src class c) (max over cores)
    moff[b][c]  = chunk offset of (b, c)'s run inside its group's msgs tile
    chg[g][c]   = chunk count of merged gather call (g, c)
    ioff[g][c]  = idx-table column offset (int16 cols of 16) of call (g, c)
    soff[b][c]  = seg-table column offset of (b, c)'s chunks (block-major)
    """
    nc = bacc.Bacc("TRN2", target_bir_lowering=False, debug=False,
                   enable_asserts=False, num_devices=NCORES,
                   num_swdge_queues=NCLS)
    xt = nc.dram_tensor("xt", [F1, SH], BF16, kind="ExternalInput")
    w1 = nc.dram_tensor("w1", [F1, F2], BF16, kind="ExternalInput")
    w2p = nc.dram_tensor("w2p", [F2, TW], mybir.dt.float32, kind="ExternalInput")
    b1b = nc.dram_tensor("b1b", [P, F2], mybir.dt.float32, kind="ExternalInput")
    b2b = nc.dram_tensor("b2b", [P, F3], mybir.dt.float32, kind="ExternalInput")
    iot = nc.dram_tensor("iot", [P, P], BF16, kind="ExternalInput")
    idn = nc.dram_tensor("idn", [P, P], mybir.dt.float32, kind="ExternalInput")
    dnv = nc.dram_tensor("dnv", [P, NB], mybir.dt.float32, kind="ExternalInput")
    idx_all = nc.dram_tensor("idx_all", [P, ni16_cols], mybir.dt.int16, kind="ExternalInput")
    seg_all = nc.dram_tensor("seg_all", [P, chtot_seg], BF16, kind="ExternalInput")
    out = nc.dram_tensor("out", [SH, F3], mybir.dt.float32, kind="ExternalOutput")

    # K-chunking of the 300-wide input features
    KCH = [(0, 128), (128, 128), (256, F1 - 256)]
    addr_space = "Shared" if SHARED else "Local"
    chgt = [sum(chg[g]) for g in range(NG)]          # group msgs chunk totals
    chgmax = max(chgt)
    schb = [sum(cnk[b]) for b in range(NB)]          # per-block chunk totals
    schmax = max(schb)

    with tile.TileContext(nc) as tc:
        with (
            tc.tile_pool(name="const", bufs=1) as cp,
            tc.tile_pool(name="sb", bufs=3) as sb,
            tc.tile_pool(name="ep", bufs=3) as ep,
            tc.tile_pool(name="ps", bufs=2, space="PSUM") as ps,
            tc.tile_pool(name="psz", bufs=2, space="PSUM") as psz,
            tc.tile_pool(name="dram", bufs=1, space="DRAM") as dp,
        ):
            # --- constants ---
            w1_t = [cp.tile([k, F2], BF16, name=f"w1c{i}")
                    for i, (_, k) in enumerate(KCH)]
            for i, (o, k) in enumerate(KCH):
                nc.sync.dma_start(out=w1_t[i][:], in_=w1[o:o + k, :])
            w2_t = cp.tile([F2, TW], mybir.dt.float32)
            nc.sync.dma_start(out=w2_t[:], in_=w2p[:])
            b1_t = cp.tile([P, F2], mybir.dt.float32)
            nc.sync.dma_start(out=b1_t[:], in_=b1b[:])
            b2_t = cp.tile([P, F3], mybir.dt.float32)
            nc.sync.dma_start(out=b2_t[:], in_=b2b[:])
            iota_t = cp.tile([P, P], BF16)
            nc.sync.dma_start(out=iota_t[:], in_=iot[:])
            ident_t = cp.tile([P, P], mybir.dt.float32)
            nc.sync.dma_start(out=ident_t[:], in_=idn[:])
            dinv_t = cp.tile([P, NB], mybir.dt.float32)
            nc.sync.dma_start(out=dinv_t[:], in_=dnv[:])
            idx_t = cp.tile([P, ni16_cols], mybir.dt.int16)
            nc.sync.dma_start(out=idx_t[:], in_=idx_all[:])
            seg_t = cp.tile([P, chtot_seg], BF16)
            nc.sync.dma_start(out=seg_t[:], in_=seg_all[:])

            # --- DRAM intermediates (tables are bf16, TW-wide rows) ---
            # Each table quarter is its own tensor written by exactly ONE
            # AllGather (Shared DRAM allows a single writer instruction).
            y_cc = dp.tile([SH, TW], BF16)
            y_q = [dp.tile([QB, TW], BF16, addr_space=addr_space,
                           name=f"y_q{c}") for c in range(NCLS)]
            y2_cc = dp.tile([SH, TW], BF16)
            y2_q = [dp.tile([QB, TW], BF16, addr_space=addr_space,
                            name=f"y2_q{c}") for c in range(NCLS)]

            def allgather_quarter(src_cc, dst_q, c):
                # Chunk-major physical table layout: quarter c of the table is
                # the 8 cores' local rows [CUMB[c], CUMB[c+1]) concatenated
                # core-major — exactly the AllGather output order.
                a, b = CUMB[c], CUMB[c + 1]
                nc.gpsimd.collective_compute(
                    "AllGather", mybir.AluOpType.bypass,
                    replica_groups=[list(range(NCORES))],
                    ins=[src_cc[a:b, :]],
                    outs=[dst_q[c][:]])

            # --- stage A: y_own = dinv * (x @ W1), grouped x loads ---
            for g in range(0, NB, GA):
                gn = min(GA, NB - g)
                xcg = [sb.tile([k, gn * P], BF16, tag=f"xcg{i}", name=f"xcg{i}_{g}")
                       for i, (_, k) in enumerate(KCH)]
                for i, (o, k) in enumerate(KCH):
                    nc.sync.dma_start(out=xcg[i][:],
                                      in_=xt[o:o + k, g * P:(g + gn) * P])
                for j in range(gn):
                    b = g + j
                    xps = ps.tile([P, F2], mybir.dt.float32, space="PSUM", tag="ya")
                    for i, (o, k) in enumerate(KCH):
                        nc.tensor.matmul(out=xps[:],
                                         lhsT=xcg[i][:, j * P:(j + 1) * P],
                                         rhs=w1_t[i][:],
                                         start=(i == 0), stop=(i == len(KCH) - 1))
                    ytl = sb.tile([P, F2], BF16, tag="ytl")
                    nc.vector.tensor_scalar_mul(out=ytl[:], in0=xps[:],
                                                scalar1=dinv_t[:, b:b + 1])
                    nc.sync.dma_start(out=y_cc[b * P:(b + 1) * P, :], in_=ytl[:])
                    for c in range(NCLS):
                        if b == AGB[c + 1] - 1:
                            allgather_quarter(y_cc, y_q, c)

            def gather_group(g, tab_q, li):
                """Issue the 4 merged gather calls for group g into a msgs tile."""
                msgs = sb.tile([P, chgmax * TW], BF16, tag="msgs",
                               name=f"msgs{li}_{g}", bufs=2)
                mo = 0
                for c in range(NCLS):
                    cg = chg[g][c]
                    if cg == 0:
                        continue
                    nidx = cg * P
                    nc.gpsimd.dma_gather(
                        out_ap=msgs[:, mo * TW:(mo + cg) * TW]
                            .rearrange("p (k f) -> p k f", f=TW),
                        in_ap=tab_q[c][:],
                        idxs_ap=idx_t[:, ioff[g][c]:ioff[g][c] + nidx // 16],
                        num_idxs=nidx, num_idxs_reg=nidx,
                        elem_size=TW, elem_step=TW,
                        single_packet=False, queue_num=c)
                    mo += cg
                return msgs

            def aggregate(b, msgs, OW, bias_t, relu, dst):
                """Aggregate one dst block: one-hot matmuls + epilogue.

                OW: output feature width (128 for layer 1, 64 for layer 2).
                """
                sb0 = soff[b][0]
                ch_b = schb[b]
                # one-hot build: the Scalar (Activation) engine materializes
                # the seg ids replicated across each chunk's 128 columns (a
                # broadcast copy on an otherwise-idle engine), then DVE does
                # an in-place is_equal against iota with BOTH operands' last
                # dim packed (2-byte, stride-1) so the 2x perf mode applies.
                oh = sb.tile([P, schmax * P], BF16, tag="oh",
                             name=f"oh{b}_{OW}", bufs=3)
                nc.scalar.activation(
                    out=oh[:, :ch_b * P].rearrange("p (c f) -> p c f", f=P),
                    in_=seg_t[:, sb0:sb0 + ch_b]
                        .rearrange("p (c one) -> p c one", one=1)
                        .to_broadcast([P, ch_b, P]),
                    func=mybir.ActivationFunctionType.Copy)
                nc.vector.tensor_tensor(
                    out=oh[:, :ch_b * P].rearrange("p (c f) -> p c f", f=P),
                    in0=oh[:, :ch_b * P].rearrange("p (c f) -> p c f", f=P),
                    in1=iota_t[:].rearrange("p (one f) -> p one f", one=1)
                        .to_broadcast([P, ch_b, P]),
                    op=mybir.AluOpType.is_equal)
                zp = psz.tile([P, OW], mybir.dt.float32, space="PSUM", tag="zp",
                              name=f"zp{b}_{OW}")
                nmm = 0
                for c in range(NCLS):
                    ohb = soff[b][c] - sb0
                    mb = moff[b][c]
                    for j in range(cnk[b][c]):
                        nmm += 1
                        nc.tensor.matmul(
                            out=zp[:],
                            lhsT=oh[:, (ohb + j) * P:(ohb + j + 1) * P],
                            rhs=msgs[:, (mb + j) * TW:(mb + j) * TW + OW],
                            start=(nmm == 1), stop=(nmm == ch_b))
                # epilogue: dst = [relu](zp * dinv + bias), one fused DVE op
                zt = ep.tile([P, OW], mybir.dt.float32, tag="zt", name=f"zt{b}_{OW}")
                nc.vector.scalar_tensor_tensor(
                    out=zt[:], in0=zp[:], scalar=dinv_t[:, b:b + 1],
                    in1=bias_t[:], op0=mybir.AluOpType.mult,
                    op1=mybir.AluOpType.add)
                if relu:
                    h = ep.tile([P, OW], mybir.dt.float32, tag="h", name=f"h{b}")
                    nc.scalar.activation(out=h[:], in_=zt[:],
                                         func=mybir.ActivationFunctionType.Relu)
                    return h
                nc.sync.dma_start(out=dst[b * P:(b + 1) * P, :], in_=zt[:])
                return None

            # --- stage C+D: layer-1 aggregation + y2 production ---
            # AG2 quarter c is emitted (at the top of a group, keeping it
            # early in the Pool queue) once its y2 range closed 7+ blocks ago.
            for g in range(NG):
                for c in range(NCLS - 1):
                    if g * G <= AGB[c + 1] + 7 < (g + 1) * G:
                        allgather_quarter(y2_cc, y2_q, c)
                msgs = gather_group(g, y_q, 1)
                for b in range(g * G, (g + 1) * G):
                    h = aggregate(b, msgs, F2, b1_t, relu=True, dst=None)
                    # y2 = dinv * (h @ W2): transpose h, matmul with padded W2
                    htp = ps.tile([P, P], mybir.dt.float32, space="PSUM", tag="htp")
                    nc.tensor.transpose(out=htp[:], in_=h[:], identity=ident_t[:])
                    ht = sb.tile([P, P], mybir.dt.float32, tag="ht")
                    nc.vector.tensor_copy(out=ht[:], in_=htp[:])
                    y2ps = ps.tile([P, TW], mybir.dt.float32, space="PSUM", tag="y2ps")
                    nc.tensor.matmul(out=y2ps[:], lhsT=ht[:], rhs=w2_t[:],
                                     start=True, stop=True)
                    y2t = sb.tile([P, TW], BF16, tag="y2t")
                    nc.vector.tensor_scalar_mul(out=y2t[:], in0=y2ps[:],
                                                scalar1=dinv_t[:, b:b + 1])
                    nc.sync.dma_start(out=y2_cc[b * P:(b + 1) * P, :], in_=y2t[:])
            allgather_quarter(y2_cc, y2_q, NCLS - 1)

            # --- stage F: layer-2 aggregation ---
            for g in range(NG):
                msgs = gather_group(g, y2_q, 2)
                for b in range(g * G, (g + 1) * G):
                    aggregate(b, msgs, F3, b2_t, relu=False, dst=out)
    nc.compile()
    return nc


def _prep_inputs(x, edge_index, W1, b1, W2, b2):
    """Host-side sharding/index prep."""
    n = x.shape[0]
    loop = np.arange(n, dtype=np.int64)
    src = np.concatenate([edge_index[0].astype(np.int64), loop])
    dst = np.concatenate([edge_index[1].astype(np.int64), loop])

    deg = np.bincount(dst, minlength=n).astype(np.float32)
    dinv = np.zeros(NPAD, dtype=np.float32)
    dinv[:n] = (np.float32(1.0) / np.sqrt(deg)).astype(np.float32)

    # physical (chunk-major) table row of each node: table chunk q holds the
    # 8 cores' local rows [CUMB[q], CUMB[q+1]) concatenated core-major, so
    # each chunked AllGather writes one contiguous range. Quarter boundaries
    # coincide with even chunk boundaries.
    cumb = np.array(CUMB)
    csz = np.diff(cumb)
    s_loc = np.arange(NPAD) % SH
    s_core = np.arange(NPAD) // SH
    q_of = np.searchsorted(cumb, s_loc, side="right") - 1
    phys = (NCORES * cumb[q_of] + s_core * csz[q_of]
            + (s_loc - cumb[q_of]))

    blk = (dst % SH) // P
    seg = (dst % SH) % P
    sp = phys[src]
    cls = sp // QB
    idx16 = sp % QB

    # sort messages by (core, block, class, src-idx)
    core = dst // SH
    order = np.lexsort((idx16, cls, blk, core))
    blk_s, cls_s = blk[order], cls[order]
    seg_s, idx_s = seg[order], idx16[order]
    core_s = core[order]

    # counts per (core, block, class); chunk grid is the max over cores
    key = (core_s * NB + blk_s) * NCLS + cls_s
    cnts = np.bincount(key, minlength=NCORES * NB * NCLS).reshape(NCORES, NB, NCLS)
    cnk = np.ceil(cnts.max(axis=0) / P).astype(np.int64)          # [NB, NCLS]
    starts = np.concatenate([[0], np.cumsum(cnts.reshape(-1))])

    # column layouts
    # msgs / idx order: group-major, class-major inside: (g, c, b, chunk)
    # seg order: block-major: (b, c, chunk)
    moff = [[0] * NCLS for _ in range(NB)]
    chg = [[0] * NCLS for _ in range(NG)]
    ioff = [[0] * NCLS for _ in range(NG)]
    soff = [[0] * NCLS for _ in range(NB)]
    io = 0
    for g in range(NG):
        mo = 0
        for c in range(NCLS):
            ioff[g][c] = io
            for b in range(g * G, (g + 1) * G):
                moff[b][c] = mo
                k = int(cnk[b, c])
                mo += k
                io += k * P // 16
            chg[g][c] = mo - (moff[g * G][c])
        # moff is relative to the group's msgs tile start (mo runs over the
        # whole group: class-major), chg counts only class c's chunks
    # fix chg computation (mo accumulates across classes)
    for g in range(NG):
        for c in range(NCLS):
            chg[g][c] = sum(int(cnk[b, c]) for b in range(g * G, (g + 1) * G))
    so = 0
    for b in range(NB):
        for c in range(NCLS):
            soff[b][c] = so
            so += int(cnk[b, c])
    chtot_seg = so
    ni16 = io

    xpad = np.zeros((NPAD, x.shape[1]), dtype=np.float32)
    xpad[:n] = x
    iota = np.tile(np.arange(P, dtype=np.float32), (P, 1)).astype(ml_dtypes.bfloat16)
    ident = np.eye(P, dtype=np.float32)
    w2pad = np.zeros((F2, TW), dtype=np.float32)
    w2pad[:, :F3] = np.asarray(W2, dtype=np.float32)
    b1bc = np.tile(np.asarray(b1, dtype=np.float32), (P, 1))
    b2bc = np.tile(np.asarray(b2, dtype=np.float32), (P, 1))

    in_maps = []
    for r in range(NCORES):
        idx_cols = np.zeros((16, ni16), dtype=np.int16)
        seg_cols = np.full((P, chtot_seg), PAD_SEG, dtype=np.float32)
        for g in range(NG):
            for c in range(NCLS):
                io2 = ioff[g][c]
                for b in range(g * G, (g + 1) * G):
                    k = int(cnk[b, c])
                    if k == 0:
                        continue
                    si = starts[(r * NB + b) * NCLS + c]
                    ei = starts[(r * NB + b) * NCLS + c + 1]
                    cnt = ei - si
                    L = k * P
                    mi = np.zeros(L, dtype=np.int16)
                    mi[:cnt] = idx_s[si:ei]
                    idx_cols[:, io2:io2 + L // 16] = mi.reshape(L // 16, 16).T
                    ms = np.full(L, PAD_SEG, dtype=np.float32)
                    ms[:cnt] = seg_s[si:ei]
                    sc = soff[b][c]
                    seg_cols[:, sc:sc + k] = ms.reshape(k, P).T
                    io2 += L // 16
        dnv = dinv[r * SH:(r + 1) * SH].reshape(NB, P).T.copy()
        in_maps.append({
            "xt": np.ascontiguousarray(xpad[r * SH:(r + 1) * SH].T)
                .astype(ml_dtypes.bfloat16),
            "w1": np.asarray(W1, dtype=np.float32).astype(ml_dtypes.bfloat16),
            "w2p": w2pad,
            "b1b": b1bc, "b2b": b2bc,
            "iot": iota, "idn": ident,
            "dnv": dnv,
            "idx_all": np.tile(idx_cols, (8, 1)),
            "seg_all": seg_cols.astype(ml_dtypes.bfloat16),
        })
    return (in_maps, cnk.tolist(), moff, chg, ioff, soff, ni16, chtot_seg)


TRACE = False          # set by test harness to capture an NTFF profile
TRACE_DIR = None       # set by test harness: where trace artifacts land
LAST_EXEC_NS = None
LAST_RES = None


def kernel(x, edge_index, W1, b1, W2, b2):
    global LAST_EXEC_NS, LAST_RES
    x = np.asarray(x, dtype=np.float32)
    edge_index = np.asarray(edge_index)
    (in_maps, cnk, moff, chg, ioff, soff, ni16, chtot_seg) = _prep_inputs(
        x, edge_index, W1, b1, W2, b2)
    nc = _build_program(cnk, moff, chg, ioff, soff, ni16, chtot_seg)
    res = run_bass_kernel_spmd(nc, in_maps, core_ids=list(range(NCORES)),
                               trace=TRACE, tmpdir=TRACE_DIR)
    LAST_EXEC_NS = res.exec_time_ns
    LAST_RES = res
    outs = [res.results[r]["out"] for r in range(NCORES)]
    return np.concatenate(outs, axis=0)[:N_NODES]


if __name__ == "__main__":
    rng = np.random.default_rng(0)
    x = rng.standard_normal((N_NODES, F1), dtype=np.float32)
    ei = rng.integers(0, N_NODES, size=(2, 3200000)).astype(np.int32)
    W1 = rng.standard_normal((F1, F2), dtype=np.float32) * (1 / math.sqrt(F1))
    b1 = np.zeros(F2, np.float32)
    W2 = rng.standard_normal((F2, F3), dtype=np.float32) * (1 / math.sqrt(F2))
    b2 = np.zeros(F3, np.float32)
    out = kernel(x=x, edge_index=ei, W1=W1, b1=b1, W2=W2, b2=b2)
    print(out.shape, out.dtype)
